# revision 32
# baseline (speedup 1.0000x reference)
"""DecoderLSTM Trainium2 kernel — 8-core data-parallel over batch.

Problem: 2-layer LSTM (H=512, B=512, T=128) where the step input is the sum of
the two layers' hidden states, followed by a 3-layer MLP head applied to the
[B, T, H] hidden-sum sequence.

Strategy (per core, B_c = 64 batch rows, zero collectives):
  - LSTM gates computed as g[B_c, 4H] with the *activations* stationary on the
    PE array ([K=128, M=64] bf16 tiles of x^T / h^T) and the *weights*
    streaming as the bf16 moving operand in 512-col chunks.
  - Layer 0 gates accumulate in PSUM partitions 0-63, layer 1 in partitions
    64-127 (col tile_position 64), so PSUM banks 4-7 stay free for the MLP.
  - All activation/state buffers are [128, *] with layer 0 in the lower and
    layer 1 in the upper partition half; cell state c stays f32.
  - The MLP head consumes the hidden-sum ring directly from SBUF (no DRAM
    round-trip) and its matmul groups are interleaved into the LSTM steps as
    PE filler, so the engine never idles long enough to downclock.
  - PE program order per step: x0 | bias1+h1 | transpose h0 | x1 |
    bias0+h0(next, half) | mlp | transpose h1 | h0(next, half) | mlp.
  - Raw bass (no Tile): explicit per-engine programs and semaphores, emitted
    from a symbolic two-pass schedule.
"""

import ml_dtypes
import numpy as np

import concourse.bass as bass
import concourse.mybir as mybir
from concourse.bass_utils import run_bass_kernel_spmd

F32 = mybir.dt.float32
BF16 = mybir.dt.bfloat16
AF = mybir.ActivationFunctionType
MUL = mybir.AluOpType.mult
ADD = mybir.AluOpType.add

NCORES = 8
B, H, T, L = 512, 512, 128, 2
BC = B // NCORES          # 64 batch rows per core
G = 4 * H                 # 2048 gate rows
KT = H // 128             # 4 K-tiles
NCH = 4                   # gate column chunks of 512
RING = 8                  # steps per ring buffer
NCHUNK = (BC * T) // 512  # 16 MLP row chunks of 512

N_PRE_LOAD = 6
N_MLP_LOAD = 7


def build_nc(reps=1):
    assert reps == 1
    nc = bass.Bass("TRN2", target_bir_lowering=False, debug=False,
                   num_devices=NCORES)

    # ---- DRAM I/O ----
    x_d = nc.dram_tensor("x", [BC, H], F32, kind="ExternalInput")
    wih_d = [nc.dram_tensor(f"wih{l}", [KT, 128, G], BF16, kind="ExternalInput") for l in range(L)]
    whh_d = [nc.dram_tensor(f"whh{l}", [KT, 128, G], BF16, kind="ExternalInput") for l in range(L)]
    bias_d = [nc.dram_tensor(f"bias{l}", [1, G], BF16, kind="ExternalInput") for l in range(L)]
    ones64_d = nc.dram_tensor("ones64", [1, 64], BF16, kind="ExternalInput")
    fc1w_d = nc.dram_tensor("fc1w", [KT, 128, 512], BF16, kind="ExternalInput")
    fc2w_d = nc.dram_tensor("fc2w", [KT, 128, 512], BF16, kind="ExternalInput")
    fc3w_d = nc.dram_tensor("fc3w", [KT, 128, 512], BF16, kind="ExternalInput")
    fc1b_d = nc.dram_tensor("fc1b", [128, 4], F32, kind="ExternalInput")
    fc2b_d = nc.dram_tensor("fc2b", [128, 4], F32, kind="ExternalInput")
    fc3b_d = nc.dram_tensor("fc3b", [1, 512], BF16, kind="ExternalInput")
    ones128_d = nc.dram_tensor("ones128", [1, 128], BF16, kind="ExternalInput")
    id64_d = nc.dram_tensor("id64", [64, 64], F32, kind="ExternalInput")
    id64b_d = nc.dram_tensor("id64b", [128, 64], BF16, kind="ExternalInput")
    out_d = nc.dram_tensor("out", [BC, T, H], F32, kind="ExternalOutput")

    # ---- SBUF map ----
    off = [(nc.sbuf_base + 63) // 64 * 64]

    def at(name, shape, dtype, align=32):
        o = (off[0] + align - 1) // align * align
        h = nc.alloc_sbuf_tensor_at(name, shape, dtype, offset=o)
        off[0] = o + int(np.prod(shape[1:])) * mybir.dt.size(dtype)
        return h

    wih = [at(f"wih{l}s", [128, KT, G], BF16) for l in range(L)]
    whh = [at(f"whh{l}s", [128, KT, G], BF16) for l in range(L)]
    fc1w = at("fc1ws", [128, KT, 512], BF16)
    fc2w = at("fc2ws", [128, KT, 512], BF16)
    fc3w = at("fc3ws", [128, KT, 512], BF16)
    biasr = [at(f"bias{l}s", [1, G], BF16) for l in range(L)]
    ones64 = at("ones64s", [1, 64], BF16)
    fc3br = at("fc3bs", [1, 512], BF16)
    ones128 = at("ones128s", [1, 128], BF16)
    id64 = at("id64s", [64, 64], F32)
    id64b = at("id64bs", [128, 64], BF16)
    fc1b = at("fc1bs", [128, 4], F32)
    fc2b = at("fc2bs", [128, 4], F32)
    ring = [at(f"ring{r}", [128, KT, RING, BC], BF16) for r in range(2)]
    sig = at("sig", [128, 1536], BF16)
    tang = at("tang", [128, 512], BF16)
    tanc = at("tanc", [128, 512], BF16)
    hnew = at("hnew", [128, 512], BF16)
    tmp = at("tmp", [128, 512], BF16)
    c_sb = at("c_sb", [128, 512], F32)
    hsumT = at("hsumT", [128, KT, BC], BF16)
    h0T = at("h0T", [128, KT, BC], BF16)
    h1T = at("h1T", [128, KT, BC], BF16)
    xT = at("xT", [128, KT, BC], BF16)
    x_sb = at("x_sb", [64, 512], F32)
    out1 = at("out1", [128, KT, 512], BF16)
    out2 = at("out2", [128, KT, 512], BF16)
    out3 = [at(f"out3_{m}", [128, 512], F32) for m in range(4)]
    assert off[0] <= nc.SBUF_PARTITION_SIZE_BYTES, off[0]

    # ---- symbolic schedules (two-pass: build op lists, then emit) ----
    val = {"pe": {}, "act": {}, "dve": {}, "dout": {}}
    cnt = {"pe": 0, "act": 0, "dve": 0, "dout": 0}
    progs = {"pe": [], "act": [], "dve": [], "sync": []}

    def w(eng, sem, key):
        progs[eng].append(("w", sem, key))

    def op(eng, fn, sem=None, key=None, n=1):
        if sem is not None:
            cnt[sem] += n
            if key is not None:
                assert key not in val[sem], key
                val[sem][key] = cnt[sem]
        progs[eng].append(("o", fn, sem, n))

    def opi(eng, fn, sem, keys):
        """fn embeds len(keys) then_inc(sem) calls itself, in order."""
        for k in keys:
            cnt[sem] += 1
            assert k not in val[sem], k
            val[sem][k] = cnt[sem]
        progs[eng].append(("o", fn, None, 0))

    # --- PSUM layout (built at emit time; descriptors here) ---
    # G0: [0:64, 0:2048]   G1: [64:128, 0:2048]
    # psAB: banks 4,5 ([:, 2048:2560], [:, 2560:3072])
    # Tp32: [:, 3072:3328] f32 (x prologue)
    # TpB[l]: [:, 3328+128*l : ...] bitcast bf16 [128, 256]

    lsl = [slice(0, 64), slice(64, 128)]    # layer partition slices

    # ================= PE program =================
    def pe_bias(l):
        def f(e, P):
            gp = P["G"][l]
            for c in range(NCH):
                cs = slice(512 * c, 512 * (c + 1))
                mm = e.matmul(gp[:, cs], ones64.ap(), biasr[l].ap()[:, cs],
                              start=True, stop=False)
            return mm
        return f

    def pe_h(l, chunks):
        hstat = h0T if l == 0 else h1T
        def f(e, P):
            gp = P["G"][l]
            for c in chunks:
                cs = slice(512 * c, 512 * (c + 1))
                for k in range(KT):
                    mm = e.matmul(gp[:, cs], hstat.ap()[:, k, :],
                                  whh[l].ap()[:, k, cs], start=False, stop=False)
            return mm
        return f

    def pe_x(l, t, chunks):
        xstat = h0T if l == 1 else (xT if t == 0 else h1T)
        def f(e, P):
            gp = P["G"][l]
            for c in chunks:
                cs = slice(512 * c, 512 * (c + 1))
                for k in range(KT):
                    mm = e.matmul(gp[:, cs], xstat.ap()[:, k, :],
                                  wih[l].ap()[:, k, cs],
                                  start=False, stop=(k == KT - 1))
            return mm
        return f

    def x_gen(l, t):
        """x_group as a generator; chunk order f,g,i,o; incs pe per chunk."""
        xstat = h0T if l == 1 else (xT if t == 0 else h1T)
        def gen(e, P):
            gp = P["G"][l]
            for c in (1, 3, 0, 2):          # f, g, i, o
                cs = slice(512 * c, 512 * (c + 1))
                for k in range(KT):
                    mm = e.matmul(gp[:, cs], xstat.ap()[:, k, :],
                                  wih[l].ap()[:, k, cs],
                                  start=False, stop=(k == KT - 1))
                    if k == KT - 1:
                        mm.then_inc(P["sems"]["pe"], 1)
                    yield
        return gen

    def bh_gen(l, chunks, with_bias=False):
        """recurrent group as a generator (no incs)."""
        hstat = h0T if l == 0 else h1T
        def gen(e, P):
            gp = P["G"][l]
            for c in chunks:
                cs = slice(512 * c, 512 * (c + 1))
                for k in range(KT):
                    e.matmul(gp[:, cs], hstat.ap()[:, k, :],
                             whh[l].ap()[:, k, cs], start=False, stop=False)
                    yield
        return gen

    def inter(ga, gb):
        """Emit two matmul streams alternately (col-group concurrency)."""
        def f(e, P):
            its = [ga(e, P), gb(e, P)]
            while its:
                for it in list(its):
                    try:
                        next(it)
                    except StopIteration:
                        its.remove(it)
            return None
        return f

    def pe_transpose(l):
        def f(e, P):
            for c in range(4):
                mm = e.transpose(P["TpB"][l][:, 64 * c:64 * (c + 1)],
                                 hnew.ap()[lsl[l], 128 * c:128 * (c + 1)],
                                 id64b.ap()[lsl[l], :])
            return mm
        return f

    def pe_xpro():
        def f(e, P):
            for c in range(4):
                mm = e.transpose(P["Tp32"][:, 64 * c:64 * (c + 1)],
                                 x_sb.ap()[:, 128 * c:128 * (c + 1)], id64.ap())
            return mm
        return f

    def pe_fc1(j, m):
        def f(e, P):
            ps = P["ps"][m % 2]
            for k in range(KT):
                mm = e.matmul(ps, fc1w.ap()[:, k, 128 * m:128 * (m + 1)],
                              ring[j % 2].ap()[:, k, :, :],
                              start=(k == 0), stop=(k == KT - 1))
            return mm
        return f

    def pe_fc2(j, m):
        def f(e, P):
            ps = P["ps"][m % 2]
            for k in range(KT):
                mm = e.matmul(ps, fc2w.ap()[:, k, 128 * m:128 * (m + 1)],
                              out1.ap()[:, k, :], start=(k == 0), stop=(k == KT - 1))
            return mm
        return f

    def pe_fc3(j, m):
        def f(e, P):
            ps = P["ps"][m % 2]
            e.matmul(ps, ones128.ap(), fc3br.ap(), start=True, stop=False)
            for k in range(KT):
                mm = e.matmul(ps, out2.ap()[:, k, 128 * m:128 * (m + 1)],
                              fc3w.ap()[:, k, :], start=False, stop=(k == KT - 1))
            return mm
        return f

    # MLP group table: 12 groups per chunk, placed at 2 slots per step in the
    # window steps 8j+8 .. 8j+13 (chunk 15 trails after the loop).
    def mlp_group(j, g):
        kind, m = ("fc1", "fc2", "fc3")[g // 4], g % 4
        if kind == "fc1":
            w("pe", "dve", f"hsum@{8 * j + 7}")
            if j == 0 and g == 0:
                w("pe", "mlp_in", 16 * N_MLP_LOAD)
            if m == 0 and j > 0:
                w("pe", "dve", f"o3m2@{j - 1}")
            if m == 1 and j > 0:
                w("pe", "dve", f"o3m3@{j - 1}")
            if m == 2:
                w("pe", "act", f"relu1m0@{j}")
            if m == 3:
                w("pe", "act", f"relu1m1@{j}")
            op("pe", pe_fc1(j, m), "pe", f"fc1m{m}@{j}")
        elif kind == "fc2":
            w("pe", "act", f"relu1m3@{j}")
            if m == 2:
                w("pe", "act", f"relu2m0@{j}")
            if m == 3:
                w("pe", "act", f"relu2m1@{j}")
            op("pe", pe_fc2(j, m), "pe", f"fc2m{m}@{j}")
        else:
            w("pe", "act", f"relu2m3@{j}")
            if m == 2:
                w("pe", "dve", f"o3m0@{j}")
            if m == 3:
                w("pe", "dve", f"o3m1@{j}")
            op("pe", pe_fc3(j, m), "pe", f"fc3m{m}@{j}")

    def mlp_slots(t):
        """(chunk, group) list for the two insertion points of step t."""
        j, s = (t - 8) // 8, (t - 8) % 8
        if t >= 8 and j < NCHUNK - 1 and s < 6:
            return [(j, 2 * s), (j, 2 * s + 1)]
        return []

    # prologue: needs only the small pre-group (x, identities, biases)
    w("pe", "pre_in", 16 * N_PRE_LOAD)
    op("pe", pe_xpro(), "pe", "xTp")

    for t in range(T):
        slots = mlp_slots(t)
        xkeys0 = [f"cf_0@{t}", f"cg_0@{t}", f"ci_0@{t}", f"co_0@{t}"]
        xkeys1 = [f"cf_1@{t}", f"cg_1@{t}", f"ci_1@{t}", f"co_1@{t}"]
        # [A] L0 x-part (moving wih0, stationary h1T; Wc0 carries the h0 sum)
        if t == 0:
            w("pe", "dve", "xT")
            op("pe", pe_bias(0))
            w("pe", "wih0_in", 16)
        else:
            w("pe", "dve", f"hT1@{t - 1}")
        opi("pe", inter(x_gen(0, t), bh_gen(1, ())), "pe", xkeys0)
        # [B] L1 bias + recurrent chunks 0,1
        if t == 0:
            op("pe", pe_bias(1))
        else:
            w("pe", "act", f"sigo1@{t - 1}")
            op("pe", pe_bias(1))
            if t == 1:
                w("pe", "whh1_in", 16)
            op("pe", pe_h(1, (0, 1)))
        # [C] transpose h0
        w("pe", "dve", f"h0@{t}")
        op("pe", pe_transpose(0), "pe", f"T0@{t}")
        # L1 recurrent chunks 2,3 — deps long met; hides the h0T copy
        if t > 0:
            op("pe", pe_h(1, (2, 3)))
        # [D] L1 x-part
        w("pe", "dve", f"hT0@{t}")
        if t == 0:
            w("pe", "wih1_in", 16)
        opi("pe", inter(x_gen(1, t), bh_gen(0, ())), "pe", xkeys1)
        # [F1] next-step L0 bias + h chunks 0,1
        if t + 1 < T:
            w("pe", "act", f"sigo0@{t}")
            if t == 0:
                w("pe", "whh0_in", 16)
            op("pe", pe_bias(0))
            op("pe", pe_h(0, (0, 1)))
        if slots:
            mlp_group(*slots[0])
        # [E] transpose h1
        w("pe", "dve", f"h1@{t}")
        op("pe", pe_transpose(1), "pe", f"T1@{t}")
        # [F2] next-step L0 h chunks 2,3
        if t + 1 < T:
            op("pe", pe_h(0, (2, 3)))
        if slots:
            mlp_group(*slots[1])
    for g in range(12):
        mlp_group(NCHUNK - 1, g)

    # ================= ACT program =================
    def act_sig(l, lo, hi):
        def f(e, P):
            return e.activation(sig.ap()[lsl[l], lo:hi], P["G"][l][:, lo:hi],
                                AF.Sigmoid)
        return f

    def act_tan(l, src):
        def f(e, P):
            if src == "g":
                return e.activation(tang.ap()[lsl[l], :], P["G"][l][:, 1536:2048],
                                    AF.Tanh)
            return e.activation(tanc.ap()[lsl[l], :], c_sb.ap()[lsl[l], :], AF.Tanh)
        return f

    def act_relu(which, m):
        dst, bias_t = (out1, fc1b) if which == 1 else (out2, fc2b)
        def f(e, P):
            return e.activation(dst.ap()[:, m, :], P["ps"][m % 2], AF.Relu,
                                bias=bias_t.ap()[:, m:m + 1])
        return f

    for t in range(T):
        for l in range(L):
            w("act", "pe", f"cf_{l}@{t}")
            op("act", act_sig(l, 512, 1024), "act", f"sigf{l}@{t}")
            w("act", "pe", f"cg_{l}@{t}")
            op("act", act_tan(l, "g"), "act", f"tang{l}@{t}")
            w("act", "pe", f"ci_{l}@{t}")
            op("act", act_sig(l, 0, 512), "act", f"sigi{l}@{t}")
            w("act", "pe", f"co_{l}@{t}")
            op("act", act_sig(l, 1024, 1536), "act", f"sigo{l}@{t}")
            w("act", "dve", f"c{l}@{t}")
            op("act", act_tan(l, "c"), "act", f"tanc{l}@{t}")
        for j, g in mlp_slots(t):
            if g < 8:
                which, m = (1, g) if g < 4 else (2, g - 4)
                w("act", "pe", f"fc{which}m{m}@{j}")
                op("act", act_relu(which, m), "act", f"relu{which}m{m}@{j}")
    for g in range(8):
        j = NCHUNK - 1
        which, m = (1, g) if g < 4 else (2, g - 4)
        w("act", "pe", f"fc{which}m{m}@{j}")
        op("act", act_relu(which, m), "act", f"relu{which}m{m}@{j}")

    # ================= DVE program =================
    def dve_tt(dst, a, b, alu, l=None, dsts=None):
        def f(e, P):
            s = lsl[l] if l is not None else slice(None)
            d = dst.ap()[s, :] if dsts is None else dsts
            return e.tensor_tensor(d, a, b, alu)
        return f

    def dve_xT():
        def f(e, P):
            return e.tensor_copy(xT.ap().rearrange("p k b -> p (k b)"), P["Tp32"])
        return f

    def dve_hT(l):
        hT = h0T if l == 0 else h1T
        def f(e, P):
            return e.tensor_copy(hT.ap().rearrange("p k b -> p (k b)"), P["TpB"][l])
        return f

    def dve_copy(dst_fn):
        def f(e, P):
            d, s = dst_fn(P)
            return e.tensor_copy(d, s)
        return f

    w("dve", "pe", "xTp")
    op("dve", dve_xT(), "dve", "xT")
    for t in range(T):
        for l in range(L):
            s = lsl[l]
            if t > 0:
                w("dve", "act", f"sigf{l}@{t}")
                op("dve", dve_tt(c_sb, c_sb.ap()[s, :], sig.ap()[s, 512:1024],
                                 MUL, l=l))
            w("dve", "act", f"sigi{l}@{t}")
            op("dve", dve_tt(tmp, sig.ap()[s, 0:512], tang.ap()[s, :], MUL, l=l))
            if t == 0:
                op("dve", dve_copy(lambda P, s=s: (c_sb.ap()[s, :], tmp.ap()[s, :])),
                   "dve", f"c{l}@{t}")
            else:
                op("dve", dve_tt(c_sb, c_sb.ap()[s, :], tmp.ap()[s, :], ADD, l=l),
                   "dve", f"c{l}@{t}")
            w("dve", "act", f"tanc{l}@{t}")
            op("dve", dve_tt(hnew, sig.ap()[s, 1024:1536], tanc.ap()[s, :],
                             MUL, l=l), "dve", f"h{l}@{t}")
            w("dve", "pe", f"T{l}@{t}")
            op("dve", dve_hT(l), "dve", f"hT{l}@{t}")
        op("dve", dve_tt(hsumT, h0T.ap(), h1T.ap(), ADD, dsts=hsumT.ap()))
        blk = t // RING
        if blk >= 2:
            w("dve", "pe", f"fc1m3@{blk - 2}")
        op("dve", dve_copy(lambda P, r=blk % 2, sl=t % RING:
                           (ring[r].ap()[:, :, sl, :], hsumT.ap())),
           "dve", f"hsum@{t}")
        for j, g in mlp_slots(t):
            if g >= 8:
                m = g - 8
                w("dve", "pe", f"fc3m{m}@{j}")
                if j > 0:
                    w("dve", "dout", f"out{m}@{j - 1}")
                op("dve", dve_copy(lambda P, m=m: (out3[m].ap(), P["ps"][m % 2])),
                   "dve", f"o3m{m}@{j}")
    for m in range(4):
        j = NCHUNK - 1
        w("dve", "pe", f"fc3m{m}@{j}")
        w("dve", "dout", f"out{m}@{j - 1}")
        op("dve", dve_copy(lambda P, m=m: (out3[m].ap(), P["ps"][m % 2])),
           "dve", f"o3m{m}@{j}")

    # ================= SYNC (DMA) program =================
    def s_load(dst, src, sem):
        def f(e, P):
            return e.dma_start(out=dst, in_=src)
        return (f, sem)

    loads = [
        s_load(x_sb.ap(), x_d.ap(), "pre_in"),
        s_load(id64.ap(), id64_d.ap(), "pre_in"),
        s_load(id64b.ap(), id64b_d.ap(), "pre_in"),
        s_load(ones64.ap(), ones64_d.ap(), "pre_in"),
        s_load(biasr[0].ap(), bias_d[0].ap(), "pre_in"),
        s_load(biasr[1].ap(), bias_d[1].ap(), "pre_in"),
        s_load(wih[0].ap(), wih_d[0].ap().rearrange("k p c -> p k c"), "wih0_in"),
        s_load(wih[1].ap(), wih_d[1].ap().rearrange("k p c -> p k c"), "wih1_in"),
        s_load(whh[0].ap(), whh_d[0].ap().rearrange("k p c -> p k c"), "whh0_in"),
        s_load(whh[1].ap(), whh_d[1].ap().rearrange("k p c -> p k c"), "whh1_in"),
        s_load(fc1w.ap(), fc1w_d.ap().rearrange("k p c -> p k c"), "mlp_in"),
        s_load(fc2w.ap(), fc2w_d.ap().rearrange("k p c -> p k c"), "mlp_in"),
        s_load(fc3w.ap(), fc3w_d.ap().rearrange("k p c -> p k c"), "mlp_in"),
        s_load(fc1b.ap(), fc1b_d.ap(), "mlp_in"),
        s_load(fc2b.ap(), fc2b_d.ap(), "mlp_in"),
        s_load(fc3br.ap(), fc3b_d.ap(), "mlp_in"),
        s_load(ones128.ap(), ones128_d.ap(), "mlp_in"),
    ]
    assert sum(1 for _, s in loads if s == "pre_in") == N_PRE_LOAD
    assert sum(1 for _, s in loads if s == "mlp_in") == N_MLP_LOAD

    def s_out(j, m):
        tt = 8 * j + 2 * m
        def f(e, P):
            return e.dma_start(out=out_d.ap()[:, tt:tt + 2, :]
                               .rearrange("b u h -> u b h"), in_=out3[m].ap())
        return f

    for j in range(NCHUNK):
        for m in range(4):
            w("sync", "dve", f"o3m{m}@{j}")
            op("sync", s_out(j, m), "dout", f"out{m}@{j}", n=16)

    # ================= emission =================
    with (
        nc.psum_tensor("P", [128, 4096], F32) as P_,
        nc.semaphore("pre_in") as pre_in,
        nc.semaphore("wih0_in") as wih0_in,
        nc.semaphore("wih1_in") as wih1_in,
        nc.semaphore("whh0_in") as whh0_in,
        nc.semaphore("whh1_in") as whh1_in,
        nc.semaphore("mlp_in") as mlp_in,
        nc.semaphore("dma_out") as dma_out,
        nc.semaphore("pe_s") as pe_s,
        nc.semaphore("act_s") as act_s,
        nc.semaphore("dve_s") as dve_s,
        nc.Block() as block,
    ):
        Pap = P_.ap()
        P = {
            "G": [Pap[0:64, 0:2048], Pap[64:128, 0:2048]],
            "ps": [Pap[:, 2048:2560], Pap[:, 2560:3072]],
            "Tp32": Pap[0:128, 3072:3328],
            "TpB": [Pap[0:128, 3328 + 128 * i:3456 + 128 * i].bitcast(BF16)
                    for i in range(2)],
        }
        sems = {"pe": pe_s, "act": act_s, "dve": dve_s, "dout": dma_out,
                "pre_in": pre_in, "wih0_in": wih0_in, "wih1_in": wih1_in,
                "whh0_in": whh0_in, "whh1_in": whh1_in, "mlp_in": mlp_in}
        P["sems"] = sems

        def emit(e, prog):
            for item in prog:
                if item[0] == "w":
                    _, sem, key = item
                    v = key if isinstance(key, int) else val[sem][key]
                    e.wait_ge(sems[sem], v)
                else:
                    _, fn, sem, n = item
                    inst = fn(e, P)
                    if sem is not None:
                        inst.then_inc(sems[sem], n)

        @block.sync
        def _(sync):
            for fn, sem in loads:
                fn(sync, P).then_inc(sems[sem], 16)
            emit(sync, progs["sync"])

        @block.tensor
        def _(tensor):
            emit(tensor, progs["pe"])

        @block.scalar
        def _(scalar):
            emit(scalar, progs["act"])

        @block.vector
        def _(vector):
            emit(vector, progs["dve"])

    return nc


_PERM = None


def _gate_perm():
    # torch gate order (i, f, g, o) -> our column order (i, f, o, g)
    global _PERM
    if _PERM is None:
        i = np.arange(512)
        _PERM = np.concatenate([i, 512 + i, 1536 + i, 1024 + i])
    return _PERM


def _prep_inputs(x, W_ih, W_hh, b_ih, b_hh, fc1_w, fc1_b, fc2_w, fc2_b, fc3_w, fc3_b):
    perm = _gate_perm()
    bf = ml_dtypes.bfloat16
    common = {}
    for l in range(L):
        wt = np.ascontiguousarray(W_ih[l][perm].T)          # [512, 2048]
        common[f"wih{l}"] = wt.reshape(KT, 128, G).astype(bf)
        # layer 0's "recurrent" weights carry W_ih0 + W_hh0: its step input is
        # hsum = h0 + h1, split as (W_ih0+W_hh0)@h0 + W_ih0@h1
        wsrc = W_ih[0] + W_hh[0] if l == 0 else W_hh[1]
        wt = np.ascontiguousarray(wsrc[perm].T)
        common[f"whh{l}"] = wt.reshape(KT, 128, G).astype(bf)
        common[f"bias{l}"] = (b_ih[l] + b_hh[l])[perm].reshape(1, G).astype(bf)
    common["fc1w"] = np.ascontiguousarray(fc1_w.T).reshape(KT, 128, 512).astype(bf)
    common["fc2w"] = np.ascontiguousarray(fc2_w.T).reshape(KT, 128, 512).astype(bf)
    common["fc3w"] = np.ascontiguousarray(fc3_w.T).reshape(KT, 128, 512).astype(bf)
    common["fc1b"] = np.ascontiguousarray(fc1_b.reshape(4, 128).T)
    common["fc2b"] = np.ascontiguousarray(fc2_b.reshape(4, 128).T)
    common["fc3b"] = fc3_b.reshape(1, 512).astype(bf)
    common["ones64"] = np.ones((1, 64), bf)
    common["ones128"] = np.ones((1, 128), bf)
    common["id64"] = np.eye(64, dtype=np.float32)
    eye = np.eye(64)
    common["id64b"] = np.concatenate([eye, eye], axis=0).astype(bf)
    in_maps = []
    for c in range(NCORES):
        m = dict(common)
        m["x"] = np.ascontiguousarray(x[BC * c:BC * (c + 1)])
        in_maps.append(m)
    return in_maps


_NC_CACHE = None


def kernel(**inputs):
    global _NC_CACHE
    if _NC_CACHE is None:
        _NC_CACHE = build_nc()
    nc = _NC_CACHE
    in_maps = _prep_inputs(**{k: np.asarray(v) for k, v in inputs.items()})
    res = run_bass_kernel_spmd(nc, in_maps, core_ids=list(range(NCORES)))
    out = np.concatenate([res.results[c]["out"] for c in range(NCORES)], axis=0)
    return out.astype(np.float32)


# revision 33
# speedup vs baseline: 1.1443x; 1.1443x over previous
"""DecoderLSTM Trainium2 kernel — 8-core data-parallel over batch.

Problem: 2-layer LSTM (H=512, B=512, T=128) where the step input is the sum of
the two layers' hidden states, followed by a 3-layer MLP head applied to the
[B, T, H] hidden-sum sequence.

Strategy (per core, B_c = 64 batch rows, zero collectives):
  - LSTM gates computed as g[B_c, 4H] with the *activations* stationary on the
    PE array ([K=128, M=64] bf16 tiles of x^T / h^T) and the *weights*
    streaming as the bf16 moving operand in 512-col chunks.
  - Layer 0 gates accumulate in PSUM partitions 0-63, layer 1 in partitions
    64-127 (col tile_position 64), so PSUM banks 4-7 stay free for the MLP.
  - All activation/state buffers are [128, *] with layer 0 in the lower and
    layer 1 in the upper partition half; cell state c stays f32.
  - The MLP head consumes the hidden-sum ring directly from SBUF (no DRAM
    round-trip) and its matmul groups are interleaved into the LSTM steps as
    PE filler, so the engine never idles long enough to downclock.
  - PE program order per step: x0 | bias1+h1 | transpose h0 | x1 |
    bias0+h0(next, half) | mlp | transpose h1 | h0(next, half) | mlp.
  - Raw bass (no Tile): explicit per-engine programs and semaphores, emitted
    from a symbolic two-pass schedule.
"""

import ml_dtypes
import numpy as np

import concourse.bass as bass
import concourse.mybir as mybir
from concourse.bass_utils import run_bass_kernel_spmd

F32 = mybir.dt.float32
BF16 = mybir.dt.bfloat16
AF = mybir.ActivationFunctionType
MUL = mybir.AluOpType.mult
ADD = mybir.AluOpType.add

NCORES = 8
B, H, T, L = 512, 512, 128, 2
BC = B // NCORES          # 64 batch rows per core
G = 4 * H                 # 2048 gate rows
KT = H // 128             # 4 K-tiles
NCH = 4                   # gate column chunks of 512
RING = 8                  # steps per ring buffer
NCHUNK = (BC * T) // 512  # 16 MLP row chunks of 512

N_PRE_LOAD = 6
N_MLP_LOAD = 7


def build_nc(reps=1):
    assert reps == 1
    nc = bass.Bass("TRN2", target_bir_lowering=False, debug=False,
                   num_devices=NCORES)

    # ---- DRAM I/O ----
    x_d = nc.dram_tensor("x", [BC, H], F32, kind="ExternalInput")
    wih_d = [nc.dram_tensor(f"wih{l}", [KT, 128, G], BF16, kind="ExternalInput") for l in range(L)]
    whh_d = [nc.dram_tensor(f"whh{l}", [KT, 128, G], BF16, kind="ExternalInput") for l in range(L)]
    bias_d = [nc.dram_tensor(f"bias{l}", [1, G], BF16, kind="ExternalInput") for l in range(L)]
    ones64_d = nc.dram_tensor("ones64", [1, 64], BF16, kind="ExternalInput")
    fc1w_d = nc.dram_tensor("fc1w", [KT, 128, 512], BF16, kind="ExternalInput")
    fc2w_d = nc.dram_tensor("fc2w", [KT, 128, 512], BF16, kind="ExternalInput")
    fc3w_d = nc.dram_tensor("fc3w", [KT, 128, 512], BF16, kind="ExternalInput")
    fc1b_d = nc.dram_tensor("fc1b", [128, 4], F32, kind="ExternalInput")
    fc2b_d = nc.dram_tensor("fc2b", [128, 4], F32, kind="ExternalInput")
    fc3b_d = nc.dram_tensor("fc3b", [1, 512], BF16, kind="ExternalInput")
    ones128_d = nc.dram_tensor("ones128", [1, 128], BF16, kind="ExternalInput")
    id64_d = nc.dram_tensor("id64", [64, 64], F32, kind="ExternalInput")
    id64b_d = nc.dram_tensor("id64b", [128, 64], BF16, kind="ExternalInput")
    out_d = nc.dram_tensor("out", [BC, T, H], F32, kind="ExternalOutput")

    # ---- SBUF map ----
    off = [(nc.sbuf_base + 63) // 64 * 64]

    def at(name, shape, dtype, align=32):
        o = (off[0] + align - 1) // align * align
        h = nc.alloc_sbuf_tensor_at(name, shape, dtype, offset=o)
        off[0] = o + int(np.prod(shape[1:])) * mybir.dt.size(dtype)
        return h

    wih = [at(f"wih{l}s", [128, KT, G], BF16) for l in range(L)]
    whh = [at(f"whh{l}s", [128, KT, G], BF16) for l in range(L)]
    fc1w = at("fc1ws", [128, KT, 512], BF16)
    fc2w = at("fc2ws", [128, KT, 512], BF16)
    fc3w = at("fc3ws", [128, KT, 512], BF16)
    biasr = [at(f"bias{l}s", [1, G], BF16) for l in range(L)]
    ones64 = at("ones64s", [1, 64], BF16)
    fc3br = at("fc3bs", [1, 512], BF16)
    ones128 = at("ones128s", [1, 128], BF16)
    id64 = at("id64s", [64, 64], F32)
    id64b = at("id64bs", [128, 64], BF16)
    fc1b = at("fc1bs", [128, 4], F32)
    fc2b = at("fc2bs", [128, 4], F32)
    ring = [at(f"ring{r}", [128, KT, RING, BC], BF16) for r in range(2)]
    sig = at("sig", [128, 1536], BF16)
    tang = at("tang", [128, 512], BF16)
    tanc = at("tanc", [128, 512], BF16)
    hnew = at("hnew", [128, 512], BF16)
    tmp = at("tmp", [128, 512], BF16)
    c_sb = at("c_sb", [128, 512], F32)
    hsumT = at("hsumT", [128, KT, BC], BF16)
    h0T = at("h0T", [128, KT, BC], BF16)
    h1T = at("h1T", [128, KT, BC], BF16)
    xT = at("xT", [128, KT, BC], BF16)
    x_sb = at("x_sb", [64, 512], F32)
    out1 = at("out1", [128, KT, 512], BF16)
    out2 = at("out2", [128, KT, 512], BF16)
    out3 = [at(f"out3_{m}", [128, 512], F32) for m in range(4)]
    assert off[0] <= nc.SBUF_PARTITION_SIZE_BYTES, off[0]

    # ---- symbolic schedules (two-pass: build op lists, then emit) ----
    val = {"pe": {}, "act": {}, "dve": {}, "dout": {}}
    cnt = {"pe": 0, "act": 0, "dve": 0, "dout": 0}
    progs = {"pe": [], "act": [], "dve": [], "sync": []}

    def w(eng, sem, key):
        progs[eng].append(("w", sem, key))

    def op(eng, fn, sem=None, key=None, n=1):
        if sem is not None:
            cnt[sem] += n
            if key is not None:
                assert key not in val[sem], key
                val[sem][key] = cnt[sem]
        progs[eng].append(("o", fn, sem, n))

    def opi(eng, fn, sem, keys):
        """fn embeds len(keys) then_inc(sem) calls itself, in order."""
        for k in keys:
            cnt[sem] += 1
            assert k not in val[sem], k
            val[sem][k] = cnt[sem]
        progs[eng].append(("o", fn, None, 0))

    # --- PSUM layout (built at emit time; descriptors here) ---
    # G0: [0:64, 0:2048]   G1: [64:128, 0:2048]
    # psAB: banks 4,5 ([:, 2048:2560], [:, 2560:3072])
    # Tp32: [:, 3072:3328] f32 (x prologue)
    # TpB[l]: [:, 3328+128*l : ...] bitcast bf16 [128, 256]

    lsl = [slice(0, 64), slice(64, 128)]    # layer partition slices

    # ================= PE program =================
    def pe_bias(l):
        def f(e, P):
            gp = P["G"][l]
            for c in range(NCH):
                cs = slice(512 * c, 512 * (c + 1))
                mm = e.matmul(gp[:, cs], ones64.ap(), biasr[l].ap()[:, cs],
                              start=True, stop=False)
            return mm
        return f

    def pe_h(l, chunks):
        hstat = h0T if l == 0 else h1T
        def f(e, P):
            gp = P["G"][l]
            for c in chunks:
                cs = slice(512 * c, 512 * (c + 1))
                for k in range(KT):
                    mm = e.matmul(gp[:, cs], hstat.ap()[:, k, :],
                                  whh[l].ap()[:, k, cs], start=False, stop=False)
            return mm
        return f

    def pe_x(l, t, chunks):
        xstat = h0T if l == 1 else (xT if t == 0 else h1T)
        def f(e, P):
            gp = P["G"][l]
            for c in chunks:
                cs = slice(512 * c, 512 * (c + 1))
                for k in range(KT):
                    mm = e.matmul(gp[:, cs], xstat.ap()[:, k, :],
                                  wih[l].ap()[:, k, cs],
                                  start=False, stop=(k == KT - 1))
            return mm
        return f

    def x_gen(l, t):
        """x_group as a generator; incs pe after chunks 1, 2, 3."""
        xstat = h0T if l == 1 else (xT if t == 0 else h1T)
        def gen(e, P):
            gp = P["G"][l]
            n = 0
            for c in range(NCH):
                cs = slice(512 * c, 512 * (c + 1))
                for k in range(KT):
                    mm = e.matmul(gp[:, cs], xstat.ap()[:, k, :],
                                  wih[l].ap()[:, k, cs],
                                  start=False, stop=(k == KT - 1))
                    n += 1
                    if n in (8, 12, 16):
                        mm.then_inc(P["sems"]["pe"], 1)
                    yield
        return gen

    def bh_gen(l, chunks, with_bias=False):
        """recurrent group as a generator (no incs)."""
        hstat = h0T if l == 0 else h1T
        def gen(e, P):
            gp = P["G"][l]
            for c in chunks:
                cs = slice(512 * c, 512 * (c + 1))
                for k in range(KT):
                    e.matmul(gp[:, cs], hstat.ap()[:, k, :],
                             whh[l].ap()[:, k, cs], start=False, stop=False)
                    yield
        return gen

    def inter(ga, gb):
        """Emit two matmul streams alternately (col-group concurrency)."""
        def f(e, P):
            its = [ga(e, P), gb(e, P)]
            while its:
                for it in list(its):
                    try:
                        next(it)
                    except StopIteration:
                        its.remove(it)
            return None
        return f

    def pe_transpose(l):
        def f(e, P):
            for c in range(4):
                mm = e.transpose(P["TpB"][l][:, 64 * c:64 * (c + 1)],
                                 hnew.ap()[lsl[l], 128 * c:128 * (c + 1)],
                                 id64b.ap()[lsl[l], :])
            return mm
        return f

    def pe_xpro():
        def f(e, P):
            for c in range(4):
                mm = e.transpose(P["Tp32"][:, 64 * c:64 * (c + 1)],
                                 x_sb.ap()[:, 128 * c:128 * (c + 1)], id64.ap())
            return mm
        return f

    def pe_fc1(j, m):
        def f(e, P):
            ps = P["ps"][m % 2]
            for k in range(KT):
                mm = e.matmul(ps, fc1w.ap()[:, k, 128 * m:128 * (m + 1)],
                              ring[j % 2].ap()[:, k, :, :],
                              start=(k == 0), stop=(k == KT - 1))
            return mm
        return f

    def pe_fc2(j, m):
        def f(e, P):
            ps = P["ps"][m % 2]
            for k in range(KT):
                mm = e.matmul(ps, fc2w.ap()[:, k, 128 * m:128 * (m + 1)],
                              out1.ap()[:, k, :], start=(k == 0), stop=(k == KT - 1))
            return mm
        return f

    def pe_fc3(j, m):
        def f(e, P):
            ps = P["ps"][m % 2]
            e.matmul(ps, ones128.ap(), fc3br.ap(), start=True, stop=False)
            for k in range(KT):
                mm = e.matmul(ps, out2.ap()[:, k, 128 * m:128 * (m + 1)],
                              fc3w.ap()[:, k, :], start=False, stop=(k == KT - 1))
            return mm
        return f

    # MLP group table: 12 groups per chunk, placed at 2 slots per step in the
    # window steps 8j+8 .. 8j+13 (chunk 15 trails after the loop).
    def mlp_group(j, g):
        kind, m = ("fc1", "fc2", "fc3")[g // 4], g % 4
        if kind == "fc1":
            w("pe", "dve", f"hsum@{8 * j + 7}")
            if j == 0 and g == 0:
                w("pe", "mlp_in", 16 * N_MLP_LOAD)
            if m == 0 and j > 0:
                w("pe", "dve", f"o3m2@{j - 1}")
            if m == 1 and j > 0:
                w("pe", "dve", f"o3m3@{j - 1}")
            if m == 2:
                w("pe", "act", f"relu1m0@{j}")
            if m == 3:
                w("pe", "act", f"relu1m1@{j}")
            op("pe", pe_fc1(j, m), "pe", f"fc1m{m}@{j}")
        elif kind == "fc2":
            w("pe", "act", f"relu1m3@{j}")
            if m == 2:
                w("pe", "act", f"relu2m0@{j}")
            if m == 3:
                w("pe", "act", f"relu2m1@{j}")
            op("pe", pe_fc2(j, m), "pe", f"fc2m{m}@{j}")
        else:
            w("pe", "act", f"relu2m3@{j}")
            if m == 2:
                w("pe", "dve", f"o3m0@{j}")
            if m == 3:
                w("pe", "dve", f"o3m1@{j}")
            op("pe", pe_fc3(j, m), "pe", f"fc3m{m}@{j}")

    def mlp_slots(t):
        """(chunk, group) list for the two insertion points of step t."""
        j, s = (t - 8) // 8, (t - 8) % 8
        if t >= 8 and j < NCHUNK - 1 and s < 6:
            return [(j, 2 * s), (j, 2 * s + 1)]
        return []

    # prologue: needs only the small pre-group (x, identities, biases)
    w("pe", "pre_in", 16 * N_PRE_LOAD)
    op("pe", pe_xpro(), "pe", "xTp")

    for t in range(T):
        slots = mlp_slots(t)
        xkeys0 = [f"c01_0@{t}", f"c2_0@{t}", f"c3_0@{t}"]
        xkeys1 = [f"c01_1@{t}", f"c2_1@{t}", f"c3_1@{t}"]
        # [A] L0 x-part (moving wih0, stationary h1T; Wc0 carries the h0 sum)
        if t == 0:
            w("pe", "dve", "xT")
            op("pe", pe_bias(0))
            w("pe", "wih0_in", 16)
        else:
            w("pe", "dve", f"hT1@{t - 1}")
        opi("pe", inter(x_gen(0, t), bh_gen(1, ())), "pe", xkeys0)
        # [B] L1 bias + recurrent chunks 0,1
        if t == 0:
            op("pe", pe_bias(1))
        else:
            w("pe", "act", f"sigo1@{t - 1}")
            op("pe", pe_bias(1))
            if t == 1:
                w("pe", "whh1_in", 16)
            op("pe", pe_h(1, (0, 1)))
        # [C] transpose h0
        w("pe", "dve", f"h0@{t}")
        op("pe", pe_transpose(0), "pe", f"T0@{t}")
        # L1 recurrent chunks 2,3 — deps long met; hides the h0T copy
        if t > 0:
            op("pe", pe_h(1, (2, 3)))
        # [D] L1 x-part
        w("pe", "dve", f"hT0@{t}")
        if t == 0:
            w("pe", "wih1_in", 16)
        opi("pe", inter(x_gen(1, t), bh_gen(0, ())), "pe", xkeys1)
        # [F1] next-step L0 bias + h chunks 0,1
        if t + 1 < T:
            w("pe", "act", f"sigo0@{t}")
            if t == 0:
                w("pe", "whh0_in", 16)
            op("pe", pe_bias(0))
            op("pe", pe_h(0, (0, 1)))
        if slots:
            mlp_group(*slots[0])
        # [E] transpose h1
        w("pe", "dve", f"h1@{t}")
        op("pe", pe_transpose(1), "pe", f"T1@{t}")
        # [F2] next-step L0 h chunks 2,3
        if t + 1 < T:
            op("pe", pe_h(0, (2, 3)))
        if slots:
            mlp_group(*slots[1])
    for g in range(12):
        mlp_group(NCHUNK - 1, g)

    # ================= ACT program =================
    def act_sig(l, lo, hi):
        def f(e, P):
            return e.activation(sig.ap()[lsl[l], lo:hi], P["G"][l][:, lo:hi],
                                AF.Sigmoid)
        return f

    def act_tan(l, src):
        def f(e, P):
            if src == "g":
                return e.activation(tang.ap()[lsl[l], :], P["G"][l][:, 1536:2048],
                                    AF.Tanh)
            return e.activation(tanc.ap()[lsl[l], :], c_sb.ap()[lsl[l], :], AF.Tanh)
        return f

    def act_relu(which, m):
        dst, bias_t = (out1, fc1b) if which == 1 else (out2, fc2b)
        def f(e, P):
            return e.activation(dst.ap()[:, m, :], P["ps"][m % 2], AF.Relu,
                                bias=bias_t.ap()[:, m:m + 1])
        return f

    for t in range(T):
        for l in range(L):
            w("act", "pe", f"c01_{l}@{t}")
            op("act", act_sig(l, 0, 1024), "act", f"sigif{l}@{t}")
            w("act", "pe", f"c3_{l}@{t}")
            op("act", act_tan(l, "g"), "act", f"tang{l}@{t}")
            op("act", act_sig(l, 1024, 1536), "act", f"sigo{l}@{t}")
            w("act", "dve", f"c{l}@{t}")
            op("act", act_tan(l, "c"), "act", f"tanc{l}@{t}")
        for j, g in mlp_slots(t):
            if g < 8:
                which, m = (1, g) if g < 4 else (2, g - 4)
                w("act", "pe", f"fc{which}m{m}@{j}")
                op("act", act_relu(which, m), "act", f"relu{which}m{m}@{j}")
    for g in range(8):
        j = NCHUNK - 1
        which, m = (1, g) if g < 4 else (2, g - 4)
        w("act", "pe", f"fc{which}m{m}@{j}")
        op("act", act_relu(which, m), "act", f"relu{which}m{m}@{j}")

    # ================= DVE program =================
    def dve_tt(dst, a, b, alu, l=None, dsts=None):
        def f(e, P):
            s = lsl[l] if l is not None else slice(None)
            d = dst.ap()[s, :] if dsts is None else dsts
            return e.tensor_tensor(d, a, b, alu)
        return f

    def dve_xT():
        def f(e, P):
            return e.tensor_copy(xT.ap().rearrange("p k b -> p (k b)"), P["Tp32"])
        return f

    def dve_hT(l):
        hT = h0T if l == 0 else h1T
        def f(e, P):
            return e.tensor_copy(hT.ap().rearrange("p k b -> p (k b)"), P["TpB"][l])
        return f

    def dve_copy(dst_fn):
        def f(e, P):
            d, s = dst_fn(P)
            return e.tensor_copy(d, s)
        return f

    w("dve", "pe", "xTp")
    op("dve", dve_xT(), "dve", "xT")
    for t in range(T):
        for l in range(L):
            s = lsl[l]
            if t > 0:
                w("dve", "act", f"sigif{l}@{t}")
                op("dve", dve_tt(c_sb, c_sb.ap()[s, :], sig.ap()[s, 512:1024],
                                 MUL, l=l))
            w("dve", "act", f"tang{l}@{t}")
            op("dve", dve_tt(tmp, sig.ap()[s, 0:512], tang.ap()[s, :], MUL, l=l))
            if t == 0:
                op("dve", dve_copy(lambda P, s=s: (c_sb.ap()[s, :], tmp.ap()[s, :])),
                   "dve", f"c{l}@{t}")
            else:
                op("dve", dve_tt(c_sb, c_sb.ap()[s, :], tmp.ap()[s, :], ADD, l=l),
                   "dve", f"c{l}@{t}")
            w("dve", "act", f"tanc{l}@{t}")
            op("dve", dve_tt(hnew, sig.ap()[s, 1024:1536], tanc.ap()[s, :],
                             MUL, l=l), "dve", f"h{l}@{t}")
            w("dve", "pe", f"T{l}@{t}")
            op("dve", dve_hT(l), "dve", f"hT{l}@{t}")
        op("dve", dve_tt(hsumT, h0T.ap(), h1T.ap(), ADD, dsts=hsumT.ap()))
        blk = t // RING
        if blk >= 2:
            w("dve", "pe", f"fc1m3@{blk - 2}")
        op("dve", dve_copy(lambda P, r=blk % 2, sl=t % RING:
                           (ring[r].ap()[:, :, sl, :], hsumT.ap())),
           "dve", f"hsum@{t}")
        for j, g in mlp_slots(t):
            if g >= 8:
                m = g - 8
                w("dve", "pe", f"fc3m{m}@{j}")
                if j > 0:
                    w("dve", "dout", f"out{m}@{j - 1}")
                op("dve", dve_copy(lambda P, m=m: (out3[m].ap(), P["ps"][m % 2])),
                   "dve", f"o3m{m}@{j}")
    for m in range(4):
        j = NCHUNK - 1
        w("dve", "pe", f"fc3m{m}@{j}")
        w("dve", "dout", f"out{m}@{j - 1}")
        op("dve", dve_copy(lambda P, m=m: (out3[m].ap(), P["ps"][m % 2])),
           "dve", f"o3m{m}@{j}")

    # ================= SYNC (DMA) program =================
    def s_load(dst, src, sem):
        def f(e, P):
            return e.dma_start(out=dst, in_=src)
        return (f, sem)

    loads = [
        s_load(x_sb.ap(), x_d.ap(), "pre_in"),
        s_load(id64.ap(), id64_d.ap(), "pre_in"),
        s_load(id64b.ap(), id64b_d.ap(), "pre_in"),
        s_load(ones64.ap(), ones64_d.ap(), "pre_in"),
        s_load(biasr[0].ap(), bias_d[0].ap(), "pre_in"),
        s_load(biasr[1].ap(), bias_d[1].ap(), "pre_in"),
        s_load(wih[0].ap(), wih_d[0].ap().rearrange("k p c -> p k c"), "wih0_in"),
        s_load(wih[1].ap(), wih_d[1].ap().rearrange("k p c -> p k c"), "wih1_in"),
        s_load(whh[0].ap(), whh_d[0].ap().rearrange("k p c -> p k c"), "whh0_in"),
        s_load(whh[1].ap(), whh_d[1].ap().rearrange("k p c -> p k c"), "whh1_in"),
        s_load(fc1w.ap(), fc1w_d.ap().rearrange("k p c -> p k c"), "mlp_in"),
        s_load(fc2w.ap(), fc2w_d.ap().rearrange("k p c -> p k c"), "mlp_in"),
        s_load(fc3w.ap(), fc3w_d.ap().rearrange("k p c -> p k c"), "mlp_in"),
        s_load(fc1b.ap(), fc1b_d.ap(), "mlp_in"),
        s_load(fc2b.ap(), fc2b_d.ap(), "mlp_in"),
        s_load(fc3br.ap(), fc3b_d.ap(), "mlp_in"),
        s_load(ones128.ap(), ones128_d.ap(), "mlp_in"),
    ]
    assert sum(1 for _, s in loads if s == "pre_in") == N_PRE_LOAD
    assert sum(1 for _, s in loads if s == "mlp_in") == N_MLP_LOAD

    def s_out(j, m):
        tt = 8 * j + 2 * m
        def f(e, P):
            return e.dma_start(out=out_d.ap()[:, tt:tt + 2, :]
                               .rearrange("b u h -> u b h"), in_=out3[m].ap())
        return f

    for j in range(NCHUNK):
        for m in range(4):
            w("sync", "dve", f"o3m{m}@{j}")
            op("sync", s_out(j, m), "dout", f"out{m}@{j}", n=16)

    # ================= emission =================
    with (
        nc.psum_tensor("P", [128, 4096], F32) as P_,
        nc.semaphore("pre_in") as pre_in,
        nc.semaphore("wih0_in") as wih0_in,
        nc.semaphore("wih1_in") as wih1_in,
        nc.semaphore("whh0_in") as whh0_in,
        nc.semaphore("whh1_in") as whh1_in,
        nc.semaphore("mlp_in") as mlp_in,
        nc.semaphore("dma_out") as dma_out,
        nc.semaphore("pe_s") as pe_s,
        nc.semaphore("act_s") as act_s,
        nc.semaphore("dve_s") as dve_s,
        nc.Block() as block,
    ):
        Pap = P_.ap()
        P = {
            "G": [Pap[0:64, 0:2048], Pap[64:128, 0:2048]],
            "ps": [Pap[:, 2048:2560], Pap[:, 2560:3072]],
            "Tp32": Pap[0:128, 3072:3328],
            "TpB": [Pap[0:128, 3328 + 128 * i:3456 + 128 * i].bitcast(BF16)
                    for i in range(2)],
        }
        sems = {"pe": pe_s, "act": act_s, "dve": dve_s, "dout": dma_out,
                "pre_in": pre_in, "wih0_in": wih0_in, "wih1_in": wih1_in,
                "whh0_in": whh0_in, "whh1_in": whh1_in, "mlp_in": mlp_in}
        P["sems"] = sems

        def emit(e, prog):
            for item in prog:
                if item[0] == "w":
                    _, sem, key = item
                    v = key if isinstance(key, int) else val[sem][key]
                    e.wait_ge(sems[sem], v)
                else:
                    _, fn, sem, n = item
                    inst = fn(e, P)
                    if sem is not None:
                        inst.then_inc(sems[sem], n)

        @block.sync
        def _(sync):
            for fn, sem in loads:
                fn(sync, P).then_inc(sems[sem], 16)
            emit(sync, progs["sync"])

        @block.tensor
        def _(tensor):
            emit(tensor, progs["pe"])

        @block.scalar
        def _(scalar):
            emit(scalar, progs["act"])

        @block.vector
        def _(vector):
            emit(vector, progs["dve"])

    return nc


_PERM = None


def _gate_perm():
    # torch gate order (i, f, g, o) -> our column order (i, f, o, g)
    global _PERM
    if _PERM is None:
        i = np.arange(512)
        _PERM = np.concatenate([i, 512 + i, 1536 + i, 1024 + i])
    return _PERM


def _prep_inputs(x, W_ih, W_hh, b_ih, b_hh, fc1_w, fc1_b, fc2_w, fc2_b, fc3_w, fc3_b):
    perm = _gate_perm()
    bf = ml_dtypes.bfloat16
    common = {}
    for l in range(L):
        wt = np.ascontiguousarray(W_ih[l][perm].T)          # [512, 2048]
        common[f"wih{l}"] = wt.reshape(KT, 128, G).astype(bf)
        # layer 0's "recurrent" weights carry W_ih0 + W_hh0: its step input is
        # hsum = h0 + h1, split as (W_ih0+W_hh0)@h0 + W_ih0@h1
        wsrc = W_ih[0] + W_hh[0] if l == 0 else W_hh[1]
        wt = np.ascontiguousarray(wsrc[perm].T)
        common[f"whh{l}"] = wt.reshape(KT, 128, G).astype(bf)
        common[f"bias{l}"] = (b_ih[l] + b_hh[l])[perm].reshape(1, G).astype(bf)
    common["fc1w"] = np.ascontiguousarray(fc1_w.T).reshape(KT, 128, 512).astype(bf)
    common["fc2w"] = np.ascontiguousarray(fc2_w.T).reshape(KT, 128, 512).astype(bf)
    common["fc3w"] = np.ascontiguousarray(fc3_w.T).reshape(KT, 128, 512).astype(bf)
    common["fc1b"] = np.ascontiguousarray(fc1_b.reshape(4, 128).T)
    common["fc2b"] = np.ascontiguousarray(fc2_b.reshape(4, 128).T)
    common["fc3b"] = fc3_b.reshape(1, 512).astype(bf)
    common["ones64"] = np.ones((1, 64), bf)
    common["ones128"] = np.ones((1, 128), bf)
    common["id64"] = np.eye(64, dtype=np.float32)
    eye = np.eye(64)
    common["id64b"] = np.concatenate([eye, eye], axis=0).astype(bf)
    in_maps = []
    for c in range(NCORES):
        m = dict(common)
        m["x"] = np.ascontiguousarray(x[BC * c:BC * (c + 1)])
        in_maps.append(m)
    return in_maps


_NC_CACHE = None


def kernel(**inputs):
    global _NC_CACHE
    if _NC_CACHE is None:
        _NC_CACHE = build_nc()
    nc = _NC_CACHE
    in_maps = _prep_inputs(**{k: np.asarray(v) for k, v in inputs.items()})
    res = run_bass_kernel_spmd(nc, in_maps, core_ids=list(range(NCORES)))
    out = np.concatenate([res.results[c]["out"] for c in range(NCORES)], axis=0)
    return out.astype(np.float32)


# revision 34
# speedup vs baseline: 1.1503x; 1.0053x over previous
"""DecoderLSTM Trainium2 kernel — 8-core data-parallel over batch.

Problem: 2-layer LSTM (H=512, B=512, T=128) where the step input is the sum of
the two layers' hidden states, followed by a 3-layer MLP head applied to the
[B, T, H] hidden-sum sequence.

Strategy (per core, B_c = 64 batch rows, zero collectives):
  - LSTM gates computed as g[B_c, 4H] with the *activations* stationary on the
    PE array ([K=128, M=64] bf16 tiles of x^T / h^T) and the *weights*
    streaming as the bf16 moving operand in 512-col chunks.
  - Layer 0 gates accumulate in PSUM partitions 0-63, layer 1 in partitions
    64-127 (col tile_position 64), so PSUM banks 4-7 stay free for the MLP.
  - All activation/state buffers are [128, *] with layer 0 in the lower and
    layer 1 in the upper partition half; cell state c stays f32.
  - The MLP head consumes the hidden-sum ring directly from SBUF (no DRAM
    round-trip) and its matmul groups are interleaved into the LSTM steps as
    PE filler, so the engine never idles long enough to downclock.
  - PE program order per step: x0 | bias1+h1 | transpose h0 | x1 |
    bias0+h0(next, half) | mlp | transpose h1 | h0(next, half) | mlp.
  - Raw bass (no Tile): explicit per-engine programs and semaphores, emitted
    from a symbolic two-pass schedule.
"""

import ml_dtypes
import numpy as np

import concourse.bass as bass
import concourse.mybir as mybir
from concourse.bass_utils import run_bass_kernel_spmd

F32 = mybir.dt.float32
BF16 = mybir.dt.bfloat16
AF = mybir.ActivationFunctionType
MUL = mybir.AluOpType.mult
ADD = mybir.AluOpType.add

NCORES = 8
B, H, T, L = 512, 512, 128, 2
BC = B // NCORES          # 64 batch rows per core
G = 4 * H                 # 2048 gate rows
KT = H // 128             # 4 K-tiles
NCH = 4                   # gate column chunks of 512
RING = 8                  # steps per ring buffer
NCHUNK = (BC * T) // 512  # 16 MLP row chunks of 512

N_PRE_LOAD = 6
N_MLP_LOAD = 7


def build_nc(reps=1):
    assert reps == 1
    nc = bass.Bass("TRN2", target_bir_lowering=False, debug=False,
                   num_devices=NCORES)

    # ---- DRAM I/O ----
    x_d = nc.dram_tensor("x", [BC, H], F32, kind="ExternalInput")
    wih_d = [nc.dram_tensor(f"wih{l}", [KT, 128, G], BF16, kind="ExternalInput") for l in range(L)]
    whh_d = [nc.dram_tensor(f"whh{l}", [KT, 128, G], BF16, kind="ExternalInput") for l in range(L)]
    bias_d = [nc.dram_tensor(f"bias{l}", [1, G], BF16, kind="ExternalInput") for l in range(L)]
    ones64_d = nc.dram_tensor("ones64", [1, 64], BF16, kind="ExternalInput")
    fc1w_d = nc.dram_tensor("fc1w", [KT, 128, 512], BF16, kind="ExternalInput")
    fc2w_d = nc.dram_tensor("fc2w", [KT, 128, 512], BF16, kind="ExternalInput")
    fc3w_d = nc.dram_tensor("fc3w", [KT, 128, 512], BF16, kind="ExternalInput")
    fc1b_d = nc.dram_tensor("fc1b", [128, 4], F32, kind="ExternalInput")
    fc2b_d = nc.dram_tensor("fc2b", [128, 4], F32, kind="ExternalInput")
    fc3b_d = nc.dram_tensor("fc3b", [1, 512], BF16, kind="ExternalInput")
    ones128_d = nc.dram_tensor("ones128", [1, 128], BF16, kind="ExternalInput")
    id64_d = nc.dram_tensor("id64", [64, 64], F32, kind="ExternalInput")
    id64b_d = nc.dram_tensor("id64b", [128, 64], BF16, kind="ExternalInput")
    out_d = nc.dram_tensor("out", [BC, T, H], F32, kind="ExternalOutput")

    # ---- SBUF map ----
    off = [(nc.sbuf_base + 63) // 64 * 64]

    def at(name, shape, dtype, align=32):
        o = (off[0] + align - 1) // align * align
        h = nc.alloc_sbuf_tensor_at(name, shape, dtype, offset=o)
        off[0] = o + int(np.prod(shape[1:])) * mybir.dt.size(dtype)
        return h

    wih = [at(f"wih{l}s", [128, KT, G], BF16) for l in range(L)]
    whh = [at(f"whh{l}s", [128, KT, G], BF16) for l in range(L)]
    fc1w = at("fc1ws", [128, KT, 512], BF16)
    fc2w = at("fc2ws", [128, KT, 512], BF16)
    fc3w = at("fc3ws", [128, KT, 512], BF16)
    biasr = [at(f"bias{l}s", [1, G], BF16) for l in range(L)]
    ones64 = at("ones64s", [1, 64], BF16)
    fc3br = at("fc3bs", [1, 512], BF16)
    ones128 = at("ones128s", [1, 128], BF16)
    id64 = at("id64s", [64, 64], F32)
    id64b = at("id64bs", [128, 64], BF16)
    fc1b = at("fc1bs", [128, 4], F32)
    fc2b = at("fc2bs", [128, 4], F32)
    ring = [at(f"ring{r}", [128, KT, RING, BC], BF16) for r in range(2)]
    sig = at("sig", [128, 1536], BF16)
    tang = at("tang", [128, 512], BF16)
    tanc = at("tanc", [128, 512], BF16)
    hnew = at("hnew", [128, 512], BF16)
    tmp = at("tmp", [128, 512], BF16)
    c_sb = at("c_sb", [128, 512], F32)
    hsumT = at("hsumT", [128, KT, BC], BF16)
    h0T = at("h0T", [128, KT, BC], BF16)
    h1T = at("h1T", [128, KT, BC], BF16)
    xT = at("xT", [128, KT, BC], BF16)
    x_sb = at("x_sb", [64, 512], F32)
    out1 = at("out1", [128, KT, 512], BF16)
    out2 = at("out2", [128, KT, 512], BF16)
    out3 = [at(f"out3_{m}", [128, 512], F32) for m in range(4)]
    assert off[0] <= nc.SBUF_PARTITION_SIZE_BYTES, off[0]

    # ---- symbolic schedules (two-pass: build op lists, then emit) ----
    val = {"pe": {}, "act": {}, "dve": {}, "dout": {}}
    cnt = {"pe": 0, "act": 0, "dve": 0, "dout": 0}
    progs = {"pe": [], "act": [], "dve": [], "sync": []}

    def w(eng, sem, key):
        progs[eng].append(("w", sem, key))

    def op(eng, fn, sem=None, key=None, n=1):
        if sem is not None:
            cnt[sem] += n
            if key is not None:
                assert key not in val[sem], key
                val[sem][key] = cnt[sem]
        progs[eng].append(("o", fn, sem, n))

    def opi(eng, fn, sem, keys):
        """fn embeds len(keys) then_inc(sem) calls itself, in order."""
        for k in keys:
            cnt[sem] += 1
            assert k not in val[sem], k
            val[sem][k] = cnt[sem]
        progs[eng].append(("o", fn, None, 0))

    # --- PSUM layout (built at emit time; descriptors here) ---
    # G0: [0:64, 0:2048]   G1: [64:128, 0:2048]
    # psAB: banks 4,5 ([:, 2048:2560], [:, 2560:3072])
    # Tp32: [:, 3072:3328] f32 (x prologue)
    # TpB[l]: [:, 3328+128*l : ...] bitcast bf16 [128, 256]

    lsl = [slice(0, 64), slice(64, 128)]    # layer partition slices

    # ================= PE program =================
    def pe_bias(l):
        def f(e, P):
            gp = P["G"][l]
            for c in range(NCH):
                cs = slice(512 * c, 512 * (c + 1))
                mm = e.matmul(gp[:, cs], ones64.ap(), biasr[l].ap()[:, cs],
                              start=True, stop=False)
            return mm
        return f

    def pe_h(l, chunks):
        hstat = h0T if l == 0 else h1T
        def f(e, P):
            gp = P["G"][l]
            for c in chunks:
                cs = slice(512 * c, 512 * (c + 1))
                for k in range(KT):
                    mm = e.matmul(gp[:, cs], hstat.ap()[:, k, :],
                                  whh[l].ap()[:, k, cs], start=False, stop=False)
            return mm
        return f

    def pe_x(l, t, chunks):
        xstat = h0T if l == 1 else (xT if t == 0 else h1T)
        def f(e, P):
            gp = P["G"][l]
            for c in chunks:
                cs = slice(512 * c, 512 * (c + 1))
                for k in range(KT):
                    mm = e.matmul(gp[:, cs], xstat.ap()[:, k, :],
                                  wih[l].ap()[:, k, cs],
                                  start=False, stop=(k == KT - 1))
            return mm
        return f

    def x_gen(l, t):
        """x_group as a generator; incs pe after chunks 1, 2, 3."""
        xstat = h0T if l == 1 else (xT if t == 0 else h1T)
        def gen(e, P):
            gp = P["G"][l]
            n = 0
            for c in range(NCH):
                cs = slice(512 * c, 512 * (c + 1))
                for k in range(KT):
                    mm = e.matmul(gp[:, cs], xstat.ap()[:, k, :],
                                  wih[l].ap()[:, k, cs],
                                  start=False, stop=(k == KT - 1))
                    n += 1
                    if n in (8, 12, 16):
                        mm.then_inc(P["sems"]["pe"], 1)
                    yield
        return gen

    def bh_gen(l, chunks, with_bias=False):
        """recurrent group as a generator (no incs)."""
        hstat = h0T if l == 0 else h1T
        def gen(e, P):
            gp = P["G"][l]
            for c in chunks:
                cs = slice(512 * c, 512 * (c + 1))
                for k in range(KT):
                    e.matmul(gp[:, cs], hstat.ap()[:, k, :],
                             whh[l].ap()[:, k, cs], start=False, stop=False)
                    yield
        return gen

    def inter(ga, gb):
        """Emit two matmul streams alternately (col-group concurrency)."""
        def f(e, P):
            its = [ga(e, P), gb(e, P)]
            while its:
                for it in list(its):
                    try:
                        next(it)
                    except StopIteration:
                        its.remove(it)
            return None
        return f

    def pe_transpose(l):
        def f(e, P):
            for c in range(4):
                mm = e.transpose(P["TpB"][l][:, 64 * c:64 * (c + 1)],
                                 hnew.ap()[lsl[l], 128 * c:128 * (c + 1)],
                                 id64b.ap()[lsl[l], :])
            return mm
        return f

    def pe_xpro():
        def f(e, P):
            for c in range(4):
                mm = e.transpose(P["Tp32"][:, 64 * c:64 * (c + 1)],
                                 x_sb.ap()[:, 128 * c:128 * (c + 1)], id64.ap())
            return mm
        return f

    def pe_fc1(j, m):
        def f(e, P):
            ps = P["ps"][m % 2]
            for k in range(KT):
                mm = e.matmul(ps, fc1w.ap()[:, k, 128 * m:128 * (m + 1)],
                              ring[j % 2].ap()[:, k, :, :],
                              start=(k == 0), stop=(k == KT - 1))
            return mm
        return f

    def pe_fc2(j, m):
        def f(e, P):
            ps = P["ps"][m % 2]
            for k in range(KT):
                mm = e.matmul(ps, fc2w.ap()[:, k, 128 * m:128 * (m + 1)],
                              out1.ap()[:, k, :], start=(k == 0), stop=(k == KT - 1))
            return mm
        return f

    def pe_fc3(j, m):
        def f(e, P):
            ps = P["ps"][m % 2]
            e.matmul(ps, ones128.ap(), fc3br.ap(), start=True, stop=False)
            for k in range(KT):
                mm = e.matmul(ps, out2.ap()[:, k, 128 * m:128 * (m + 1)],
                              fc3w.ap()[:, k, :], start=False, stop=(k == KT - 1))
            return mm
        return f

    # MLP group table: 12 groups per chunk, placed at 2 slots per step in the
    # window steps 8j+8 .. 8j+13 (chunk 15 trails after the loop).
    def mlp_group(j, g):
        kind, m = ("fc1", "fc2", "fc3")[g // 4], g % 4
        if kind == "fc1":
            w("pe", "dve", f"hsum@{8 * j + 7}")
            if j == 0 and g == 0:
                w("pe", "mlp_in", 16 * N_MLP_LOAD)
            if m == 0 and j > 0:
                w("pe", "dve", f"o3m2@{j - 1}")
            if m == 1 and j > 0:
                w("pe", "dve", f"o3m3@{j - 1}")
            if m == 2:
                w("pe", "act", f"relu1m0@{j}")
            if m == 3:
                w("pe", "act", f"relu1m1@{j}")
            op("pe", pe_fc1(j, m), "pe", f"fc1m{m}@{j}")
        elif kind == "fc2":
            w("pe", "act", f"relu1m3@{j}")
            if m == 2:
                w("pe", "act", f"relu2m0@{j}")
            if m == 3:
                w("pe", "act", f"relu2m1@{j}")
            op("pe", pe_fc2(j, m), "pe", f"fc2m{m}@{j}")
        else:
            w("pe", "act", f"relu2m3@{j}")
            if m == 2:
                w("pe", "dve", f"o3m0@{j}")
            if m == 3:
                w("pe", "dve", f"o3m1@{j}")
            op("pe", pe_fc3(j, m), "pe", f"fc3m{m}@{j}")

    def mlp_slots(t):
        """point -> (chunk, group) for the insertion points of step t.

        B2 sits between [B] and [C]: only groups whose deps are >=2 steps
        old may go there (fc1m0/m1) — anything fresher stalls the recurrence.
        """
        j, s = (t - 8) // 8, (t - 8) % 8
        if not (t >= 8 and j < NCHUNK - 1):
            return {}
        if s == 0:
            return {"B2": (j, 0)}
        if s == 1:
            return {"B2": (j, 1), "P1": (j, 2), "P2": (j, 3)}
        if 2 <= s <= 5:
            return {"P1": (j, 2 * s), "P2": (j, 2 * s + 1)}
        return {}

    # prologue: needs only the small pre-group (x, identities, biases)
    w("pe", "pre_in", 16 * N_PRE_LOAD)
    op("pe", pe_xpro(), "pe", "xTp")

    for t in range(T):
        slots = mlp_slots(t)
        xkeys0 = [f"c01_0@{t}", f"c2_0@{t}", f"c3_0@{t}"]
        xkeys1 = [f"c01_1@{t}", f"c2_1@{t}", f"c3_1@{t}"]
        # [A] L0 x-part (moving wih0, stationary h1T; Wc0 carries the h0 sum)
        if t == 0:
            w("pe", "dve", "xT")
            op("pe", pe_bias(0))
            w("pe", "wih0_in", 16)
        else:
            w("pe", "dve", f"hT1@{t - 1}")
        opi("pe", inter(x_gen(0, t), bh_gen(1, ())), "pe", xkeys0)
        # [B] L1 bias + recurrent chunks 0,1
        if t == 0:
            op("pe", pe_bias(1))
        else:
            w("pe", "act", f"sigo1@{t - 1}")
            op("pe", pe_bias(1))
            if t == 1:
                w("pe", "whh1_in", 16)
            op("pe", pe_h(1, (0, 1)))
        if "B2" in slots:
            mlp_group(*slots["B2"])
        # [C] transpose h0
        w("pe", "dve", f"h0@{t}")
        op("pe", pe_transpose(0), "pe", f"T0@{t}")
        # L1 recurrent chunks 2,3 — deps long met; hides the h0T copy
        if t > 0:
            op("pe", pe_h(1, (2, 3)))
        # [D] L1 x-part
        w("pe", "dve", f"hT0@{t}")
        if t == 0:
            w("pe", "wih1_in", 16)
        opi("pe", inter(x_gen(1, t), bh_gen(0, ())), "pe", xkeys1)
        # [F1] next-step L0 bias + h chunks 0,1
        if t + 1 < T:
            w("pe", "act", f"sigo0@{t}")
            if t == 0:
                w("pe", "whh0_in", 16)
            op("pe", pe_bias(0))
            op("pe", pe_h(0, (0, 1)))
        if "P1" in slots:
            mlp_group(*slots["P1"])
        # [E] transpose h1
        w("pe", "dve", f"h1@{t}")
        op("pe", pe_transpose(1), "pe", f"T1@{t}")
        # [F2] next-step L0 h chunks 2,3
        if t + 1 < T:
            op("pe", pe_h(0, (2, 3)))
        if "P2" in slots:
            mlp_group(*slots[1] if False else slots["P2"])
    for g in range(12):
        mlp_group(NCHUNK - 1, g)

    # ================= ACT program =================
    def act_sig(l, lo, hi):
        def f(e, P):
            return e.activation(sig.ap()[lsl[l], lo:hi], P["G"][l][:, lo:hi],
                                AF.Sigmoid)
        return f

    def act_tan(l, src):
        def f(e, P):
            if src == "g":
                return e.activation(tang.ap()[lsl[l], :], P["G"][l][:, 1536:2048],
                                    AF.Tanh)
            return e.activation(tanc.ap()[lsl[l], :], c_sb.ap()[lsl[l], :], AF.Tanh)
        return f

    def act_relu(which, m):
        dst, bias_t = (out1, fc1b) if which == 1 else (out2, fc2b)
        def f(e, P):
            return e.activation(dst.ap()[:, m, :], P["ps"][m % 2], AF.Relu,
                                bias=bias_t.ap()[:, m:m + 1])
        return f

    for t in range(T):
        for l in range(L):
            w("act", "pe", f"c01_{l}@{t}")
            op("act", act_sig(l, 0, 1024), "act", f"sigif{l}@{t}")
            w("act", "pe", f"c3_{l}@{t}")
            op("act", act_tan(l, "g"), "act", f"tang{l}@{t}")
            op("act", act_sig(l, 1024, 1536), "act", f"sigo{l}@{t}")
            w("act", "dve", f"c{l}@{t}")
            op("act", act_tan(l, "c"), "act", f"tanc{l}@{t}")
        for j, g in sorted(mlp_slots(t).values(), key=lambda x: x[1]):
            if g < 8:
                which, m = (1, g) if g < 4 else (2, g - 4)
                w("act", "pe", f"fc{which}m{m}@{j}")
                op("act", act_relu(which, m), "act", f"relu{which}m{m}@{j}")
    for g in range(8):
        j = NCHUNK - 1
        which, m = (1, g) if g < 4 else (2, g - 4)
        w("act", "pe", f"fc{which}m{m}@{j}")
        op("act", act_relu(which, m), "act", f"relu{which}m{m}@{j}")

    # ================= DVE program =================
    def dve_tt(dst, a, b, alu, l=None, dsts=None):
        def f(e, P):
            s = lsl[l] if l is not None else slice(None)
            d = dst.ap()[s, :] if dsts is None else dsts
            return e.tensor_tensor(d, a, b, alu)
        return f

    def dve_xT():
        def f(e, P):
            return e.tensor_copy(xT.ap().rearrange("p k b -> p (k b)"), P["Tp32"])
        return f

    def dve_hT(l):
        hT = h0T if l == 0 else h1T
        def f(e, P):
            return e.tensor_copy(hT.ap().rearrange("p k b -> p (k b)"), P["TpB"][l])
        return f

    def dve_copy(dst_fn):
        def f(e, P):
            d, s = dst_fn(P)
            return e.tensor_copy(d, s)
        return f

    w("dve", "pe", "xTp")
    op("dve", dve_xT(), "dve", "xT")
    for t in range(T):
        for l in range(L):
            s = lsl[l]
            if t > 0:
                w("dve", "act", f"sigif{l}@{t}")
                op("dve", dve_tt(c_sb, c_sb.ap()[s, :], sig.ap()[s, 512:1024],
                                 MUL, l=l))
            w("dve", "act", f"tang{l}@{t}")
            op("dve", dve_tt(tmp, sig.ap()[s, 0:512], tang.ap()[s, :], MUL, l=l))
            if t == 0:
                op("dve", dve_copy(lambda P, s=s: (c_sb.ap()[s, :], tmp.ap()[s, :])),
                   "dve", f"c{l}@{t}")
            else:
                op("dve", dve_tt(c_sb, c_sb.ap()[s, :], tmp.ap()[s, :], ADD, l=l),
                   "dve", f"c{l}@{t}")
            w("dve", "act", f"tanc{l}@{t}")
            op("dve", dve_tt(hnew, sig.ap()[s, 1024:1536], tanc.ap()[s, :],
                             MUL, l=l), "dve", f"h{l}@{t}")
            w("dve", "pe", f"T{l}@{t}")
            op("dve", dve_hT(l), "dve", f"hT{l}@{t}")
        op("dve", dve_tt(hsumT, h0T.ap(), h1T.ap(), ADD, dsts=hsumT.ap()))
        blk = t // RING
        if blk >= 2:
            w("dve", "pe", f"fc1m3@{blk - 2}")
        op("dve", dve_copy(lambda P, r=blk % 2, sl=t % RING:
                           (ring[r].ap()[:, :, sl, :], hsumT.ap())),
           "dve", f"hsum@{t}")
        for j, g in sorted(mlp_slots(t).values(), key=lambda x: x[1]):
            if g >= 8:
                m = g - 8
                w("dve", "pe", f"fc3m{m}@{j}")
                if j > 0:
                    w("dve", "dout", f"out{m}@{j - 1}")
                op("dve", dve_copy(lambda P, m=m: (out3[m].ap(), P["ps"][m % 2])),
                   "dve", f"o3m{m}@{j}")
    for m in range(4):
        j = NCHUNK - 1
        w("dve", "pe", f"fc3m{m}@{j}")
        w("dve", "dout", f"out{m}@{j - 1}")
        op("dve", dve_copy(lambda P, m=m: (out3[m].ap(), P["ps"][m % 2])),
           "dve", f"o3m{m}@{j}")

    # ================= SYNC (DMA) program =================
    def s_load(dst, src, sem):
        def f(e, P):
            return e.dma_start(out=dst, in_=src)
        return (f, sem)

    loads = [
        s_load(x_sb.ap(), x_d.ap(), "pre_in"),
        s_load(id64.ap(), id64_d.ap(), "pre_in"),
        s_load(id64b.ap(), id64b_d.ap(), "pre_in"),
        s_load(ones64.ap(), ones64_d.ap(), "pre_in"),
        s_load(biasr[0].ap(), bias_d[0].ap(), "pre_in"),
        s_load(biasr[1].ap(), bias_d[1].ap(), "pre_in"),
        s_load(wih[0].ap(), wih_d[0].ap().rearrange("k p c -> p k c"), "wih0_in"),
        s_load(wih[1].ap(), wih_d[1].ap().rearrange("k p c -> p k c"), "wih1_in"),
        s_load(whh[0].ap(), whh_d[0].ap().rearrange("k p c -> p k c"), "whh0_in"),
        s_load(whh[1].ap(), whh_d[1].ap().rearrange("k p c -> p k c"), "whh1_in"),
        s_load(fc1w.ap(), fc1w_d.ap().rearrange("k p c -> p k c"), "mlp_in"),
        s_load(fc2w.ap(), fc2w_d.ap().rearrange("k p c -> p k c"), "mlp_in"),
        s_load(fc3w.ap(), fc3w_d.ap().rearrange("k p c -> p k c"), "mlp_in"),
        s_load(fc1b.ap(), fc1b_d.ap(), "mlp_in"),
        s_load(fc2b.ap(), fc2b_d.ap(), "mlp_in"),
        s_load(fc3br.ap(), fc3b_d.ap(), "mlp_in"),
        s_load(ones128.ap(), ones128_d.ap(), "mlp_in"),
    ]
    assert sum(1 for _, s in loads if s == "pre_in") == N_PRE_LOAD
    assert sum(1 for _, s in loads if s == "mlp_in") == N_MLP_LOAD

    def s_out(j, m):
        tt = 8 * j + 2 * m
        def f(e, P):
            return e.dma_start(out=out_d.ap()[:, tt:tt + 2, :]
                               .rearrange("b u h -> u b h"), in_=out3[m].ap())
        return f

    for j in range(NCHUNK):
        for m in range(4):
            w("sync", "dve", f"o3m{m}@{j}")
            op("sync", s_out(j, m), "dout", f"out{m}@{j}", n=16)

    # ================= emission =================
    with (
        nc.psum_tensor("P", [128, 4096], F32) as P_,
        nc.semaphore("pre_in") as pre_in,
        nc.semaphore("wih0_in") as wih0_in,
        nc.semaphore("wih1_in") as wih1_in,
        nc.semaphore("whh0_in") as whh0_in,
        nc.semaphore("whh1_in") as whh1_in,
        nc.semaphore("mlp_in") as mlp_in,
        nc.semaphore("dma_out") as dma_out,
        nc.semaphore("pe_s") as pe_s,
        nc.semaphore("act_s") as act_s,
        nc.semaphore("dve_s") as dve_s,
        nc.Block() as block,
    ):
        Pap = P_.ap()
        P = {
            "G": [Pap[0:64, 0:2048], Pap[64:128, 0:2048]],
            "ps": [Pap[:, 2048:2560], Pap[:, 2560:3072]],
            "Tp32": Pap[0:128, 3072:3328],
            "TpB": [Pap[0:128, 3328 + 128 * i:3456 + 128 * i].bitcast(BF16)
                    for i in range(2)],
        }
        sems = {"pe": pe_s, "act": act_s, "dve": dve_s, "dout": dma_out,
                "pre_in": pre_in, "wih0_in": wih0_in, "wih1_in": wih1_in,
                "whh0_in": whh0_in, "whh1_in": whh1_in, "mlp_in": mlp_in}
        P["sems"] = sems

        def emit(e, prog):
            for item in prog:
                if item[0] == "w":
                    _, sem, key = item
                    v = key if isinstance(key, int) else val[sem][key]
                    e.wait_ge(sems[sem], v)
                else:
                    _, fn, sem, n = item
                    inst = fn(e, P)
                    if sem is not None:
                        inst.then_inc(sems[sem], n)

        @block.sync
        def _(sync):
            for fn, sem in loads:
                fn(sync, P).then_inc(sems[sem], 16)
            emit(sync, progs["sync"])

        @block.tensor
        def _(tensor):
            emit(tensor, progs["pe"])

        @block.scalar
        def _(scalar):
            emit(scalar, progs["act"])

        @block.vector
        def _(vector):
            emit(vector, progs["dve"])

    return nc


_PERM = None


def _gate_perm():
    # torch gate order (i, f, g, o) -> our column order (i, f, o, g)
    global _PERM
    if _PERM is None:
        i = np.arange(512)
        _PERM = np.concatenate([i, 512 + i, 1536 + i, 1024 + i])
    return _PERM


def _prep_inputs(x, W_ih, W_hh, b_ih, b_hh, fc1_w, fc1_b, fc2_w, fc2_b, fc3_w, fc3_b):
    perm = _gate_perm()
    bf = ml_dtypes.bfloat16
    common = {}
    for l in range(L):
        wt = np.ascontiguousarray(W_ih[l][perm].T)          # [512, 2048]
        common[f"wih{l}"] = wt.reshape(KT, 128, G).astype(bf)
        # layer 0's "recurrent" weights carry W_ih0 + W_hh0: its step input is
        # hsum = h0 + h1, split as (W_ih0+W_hh0)@h0 + W_ih0@h1
        wsrc = W_ih[0] + W_hh[0] if l == 0 else W_hh[1]
        wt = np.ascontiguousarray(wsrc[perm].T)
        common[f"whh{l}"] = wt.reshape(KT, 128, G).astype(bf)
        common[f"bias{l}"] = (b_ih[l] + b_hh[l])[perm].reshape(1, G).astype(bf)
    common["fc1w"] = np.ascontiguousarray(fc1_w.T).reshape(KT, 128, 512).astype(bf)
    common["fc2w"] = np.ascontiguousarray(fc2_w.T).reshape(KT, 128, 512).astype(bf)
    common["fc3w"] = np.ascontiguousarray(fc3_w.T).reshape(KT, 128, 512).astype(bf)
    common["fc1b"] = np.ascontiguousarray(fc1_b.reshape(4, 128).T)
    common["fc2b"] = np.ascontiguousarray(fc2_b.reshape(4, 128).T)
    common["fc3b"] = fc3_b.reshape(1, 512).astype(bf)
    common["ones64"] = np.ones((1, 64), bf)
    common["ones128"] = np.ones((1, 128), bf)
    common["id64"] = np.eye(64, dtype=np.float32)
    eye = np.eye(64)
    common["id64b"] = np.concatenate([eye, eye], axis=0).astype(bf)
    in_maps = []
    for c in range(NCORES):
        m = dict(common)
        m["x"] = np.ascontiguousarray(x[BC * c:BC * (c + 1)])
        in_maps.append(m)
    return in_maps


_NC_CACHE = None


def kernel(**inputs):
    global _NC_CACHE
    if _NC_CACHE is None:
        _NC_CACHE = build_nc()
    nc = _NC_CACHE
    in_maps = _prep_inputs(**{k: np.asarray(v) for k, v in inputs.items()})
    res = run_bass_kernel_spmd(nc, in_maps, core_ids=list(range(NCORES)))
    out = np.concatenate([res.results[c]["out"] for c in range(NCORES)], axis=0)
    return out.astype(np.float32)
